# revision 1
# baseline (speedup 1.0000x reference)
"""Trainium2 Bass kernel: 3x3 same-pad conv, NCHW, B=8 CIN=COUT=16 H=W=1024 f32.

Sharding: data-parallel over batch -- 1 image per NeuronCore (8 cores).

Per-core algorithm (all on-device):
  K-partition packing: K = (hi, ci) = 8 input rows x 16 cin = 128
                       M = (ho, co) = 6 output rows x 16 cout = 96
  lhsT is a host-packed banded weight matrix: lhsT[hi*16+ci, ho*16+co] =
  W[co, ci, hi-ho, kw] for hi-ho in {0,1,2} (one [128,96] matrix per kw).
  The 3 kw taps are free-dim shifts of the rhs, PSUM-accumulated.
  H blocks of 6 output rows stream through SBUF; w is split in chunks of 512
  (one PSUM bank).  Image edges are handled by slicing (bottom), zeroed halo
  partitions (top) and shrunken tap matmuls (left/right columns).
"""

import os
import sys

import numpy as np

for _p in ("/root/.axon_site", "/root/.axon_site/_ro/trn_rl_repo",
           "/root/.axon_site/_ro/pypackages", "/opt/trn_rl_repo"):
    if os.path.isdir(_p) and _p not in sys.path:
        sys.path.append(_p)

B, CIN, COUT, H, W_IMG = 8, 16, 16, 1024, 1024
HO = 6            # output rows per h-block
NCORES = 8
CHUNK = 512       # w-chunk (one fp32 PSUM bank)
M_FULL = HO * COUT  # 96

# Module-level knobs (test.py pokes these; harness uses defaults)
TRACE = False
MM_DTYPE = "float32r"   # matmul/x dtype: "float32r" | "bfloat16" | "float32"
Y_DTYPE = "float32"     # output-path dtype: "float32" | "bfloat16"
ABLATE = frozenset()    # timing experiments: {"mm", "copy", "out", "pad"}

_CACHE = {}


def _block_plan(h):
    """Per h-block geometry: (r0, r_lo, r_hi, off, hi_cnt, ho_n)."""
    plan = []
    n_blocks = (h + HO - 1) // HO
    for b in range(n_blocks):
        r0 = b * HO
        ho_n = min(HO, h - r0)
        r_lo = max(r0 - 1, 0)
        r_hi = min(r0 + ho_n, h - 1)
        off = r_lo - (r0 - 1)      # 1 iff top block (row -1 clipped)
        hi_cnt = r_hi - r_lo + 1   # input rows loaded
        plan.append((r0, r_lo, r_hi, off, hi_cnt, ho_n))
    return plan


def _pack_variant(W, hi_cnt, off):
    """Banded lhsT for one block shape: [128, 3*96] (kw-major chunks).

    K index k = ci*hi_cnt + j   (input row j of the block's loaded rows)
    M index m = ho*COUT + co    (output row ho, always 6 rows -> M=96; rows
                                 with no valid taps are all-zero and simply
                                 not stored by the output DMA)
    value = W[co, ci, kh, kw] with kh = j + off - ho, if 0 <= j < hi_cnt.
    """
    out = np.zeros((128, 3 * M_FULL), np.float32)
    for kw in range(3):
        for ho in range(HO):
            for kh in range(3):
                j = ho + kh - off
                if not (0 <= j < hi_cnt):
                    continue
                for co in range(COUT):
                    for ci in range(CIN):
                        out[ci * hi_cnt + j, kw * M_FULL + ho * COUT + co] = W[co, ci, kh, kw]
    return out


def _pack_weights(W: np.ndarray, h: int = H) -> np.ndarray:
    """Concatenate the lhsT variants for all blocks of an h-image, plus two
    trailing all-zero columns used as the copy source for the input-tile pad
    columns (memset can't write float32r)."""
    variants = []
    seen = set()
    for (_, _, _, off, hi_cnt, _) in _block_plan(h):
        key = (hi_cnt, off)
        if key not in seen:
            seen.add(key)
            variants.append(_pack_variant(W, hi_cnt, off))
    variants.append(np.zeros((128, 2), np.float32))
    return np.ascontiguousarray(np.concatenate(variants, axis=1))


def _variant_cols(h):
    """col offset of each (hi_cnt, off) variant in the packed weights."""
    cols = {}
    base = 0
    for (_, _, _, off, hi_cnt, _) in _block_plan(h):
        key = (hi_cnt, off)
        if key not in cols:
            cols[key] = base
            base += 3 * M_FULL
    return cols, base


def _conv_body(tc, y_ap, x_ap, wp_ap, h, w_img, chunk, mm_dt, y_dt=None,
               repeat=1):
    """Emit the Tile program for one core's [CIN, h, w_img] -> [COUT, h, w_img]."""
    from contextlib import ExitStack

    import concourse.mybir as mybir

    nc = tc.nc
    f32 = mybir.dt.float32
    if y_dt is None:
        y_dt = f32

    ctx = ExitStack()
    w_pool = ctx.enter_context(tc.tile_pool(name="wts", bufs=1))
    in_pool = ctx.enter_context(tc.tile_pool(name="xin", bufs=8))
    ps_pool = ctx.enter_context(tc.tile_pool(name="ps", bufs=6, space="PSUM"))
    out_pool = ctx.enter_context(tc.tile_pool(name="yout", bufs=6))

    v_cols, w_total = _variant_cols(h)
    wt = w_pool.tile([128, w_total + 2], mm_dt)
    nc.sync.dma_start(wt[:], wp_ap[:])
    zpad = wt[:, w_total:w_total + 2]  # two all-zero f32r columns

    chunks = [(w0, min(chunk, w_img - w0)) for w0 in range(0, w_img, chunk)]

    wp2 = w_img + 2  # padded input width (zero columns at 0 and w_img+1)

    if repeat > 1:
        # Benchmark mode: run the whole conv `repeat` times in one NEFF so
        # device time dominates host-side dispatch noise.
        ctx.enter_context(tc.For_i(0, repeat, 1))

    for b_idx, (r0, r_lo, r_hi, off, hi_cnt, ho_n) in enumerate(_block_plan(h)):
        k = CIN * hi_cnt               # matmul K (contiguous partitions 0..k)
        cb = v_cols[(hi_cnt, off)]
        # alternate the two HWDGE rings (SP / ACT) so input and output
        # streams each use both rings instead of saturating one
        in_eng = nc.sync if b_idx % 2 == 0 else nc.scalar
        out_eng = nc.scalar if b_idx % 2 == 0 else nc.sync

        in_t = in_pool.tile([128, wp2], mm_dt, tag="xin")
        # zero both pad columns in one strided copy (FP32r needs full even-N
        # matmuls, so w edges are handled by data padding, not partial taps;
        # memset can't write f32r, so copy from the zero columns in wt)
        if "pad" not in ABLATE:
            nc.vector.tensor_copy(in_t[:, 0:wp2:wp2 - 1], zpad)
        if "in" not in ABLATE:
            in_eng.dma_start(in_t[0:k, 1:w_img + 1], x_ap[:, r_lo:r_hi + 1, :])

        out_t = out_pool.tile([M_FULL, w_img], y_dt, tag="yout")

        for c_idx, (w0, n) in enumerate(chunks):
            ps = ps_pool.tile([M_FULL, chunk], f32, tag="ps")
            if "mm" not in ABLATE:
                for t in range(3):
                    # out[w] += tap_t . padded[w + t]
                    nc.tensor.matmul(
                        ps[:, 0:n],
                        lhsT=wt[0:k, cb + t * M_FULL: cb + (t + 1) * M_FULL],
                        rhs=in_t[0:k, w0 + t:w0 + t + n],
                        start=(t == 0),
                        stop=(t == 2),
                    )
            if "copy" not in ABLATE:
                # drain PSUM, alternating engines to split the load
                if c_idx % 2 == 0:
                    nc.scalar.copy(out_t[:, w0:w0 + n], ps[:, 0:n])
                else:
                    nc.vector.tensor_copy(out_t[:, w0:w0 + n], ps[:, 0:n])

        if "out" not in ABLATE:
            # M is packed ho-major, so a partial bottom block stores a
            # contiguous partition prefix; DRAM AP iterates (row, channel, w).
            out_eng.dma_start(
                y_ap[:, r0:r0 + ho_n, :].rearrange("c hh w -> hh c w"),
                out_t[0:ho_n * COUT, :],
            )

    ctx.close()


def _build_bass(h, w_img, chunk, mm_dtype, repeat=1, y_dtype="float32"):
    import concourse.bacc as bacc
    import concourse.mybir as mybir
    import concourse.tile as tile

    mm_dt = getattr(mybir.dt, mm_dtype)
    y_dt = getattr(mybir.dt, y_dtype)
    nc = bacc.Bacc(trn_type="TRN2", target_bir_lowering=False, debug=False)
    _, w_total = _variant_cols(h)
    x_ap = nc.dram_tensor("x_sh", [CIN, h, w_img], mm_dt, kind="ExternalInput").ap()
    wp_ap = nc.dram_tensor("wp", [128, w_total + 2], mm_dt, kind="ExternalInput").ap()
    y_ap = nc.dram_tensor("y_sh", [COUT, h, w_img], y_dt,
                          kind="ExternalOutput").ap()
    with tile.TileContext(nc) as tc:
        _conv_body(tc, y_ap, x_ap, wp_ap, h, w_img, chunk, mm_dt, y_dt=y_dt,
                   repeat=repeat)
    nc.compile()
    return nc


def kernel(x: np.ndarray, W: np.ndarray) -> np.ndarray:
    import concourse.mybir as mybir
    from concourse import bass_utils

    x = np.ascontiguousarray(np.asarray(x, dtype=np.float32))
    W = np.asarray(W, dtype=np.float32)
    wp = _pack_weights(W, H)

    key = (H, W_IMG, CHUNK, MM_DTYPE, Y_DTYPE)
    if key not in _CACHE:
        _CACHE[key] = _build_bass(H, W_IMG, CHUNK, MM_DTYPE, y_dtype=Y_DTYPE)
    nc = _CACHE[key]

    np_in = mybir.dt.np(getattr(mybir.dt, MM_DTYPE))
    wp = np.ascontiguousarray(wp.astype(np_in))
    in_maps = [
        {"x_sh": np.ascontiguousarray(x[b].astype(np_in)), "wp": wp}
        for b in range(NCORES)
    ]
    res = bass_utils.run_bass_kernel_spmd(
        nc, in_maps, core_ids=list(range(NCORES)), trace=TRACE,
    )
    out = np.stack([res.results[b]["y_sh"] for b in range(NCORES)], axis=0)
    if out.dtype != np.float32:
        out = out.astype(np.float32)
    if TRACE:
        kernel.last_results = res
    return out

